# revision 64
# baseline (speedup 1.0000x reference)
"""Trainium2 Bass kernel for nn_Bottleneck_7911329759669 (topk_masking bottleneck).

Self-contained: builds the Bass module on first call, runs SPMD on 8 NeuronCores
(data-parallel over batch, 8 samples per core), returns the full output.

Per-sample pipeline (x: [256, 3136] fp32):
  - conv1 (1x1) as exact-f32 matmul with the spatial-saliency row (mask_w)
    fused as output column 64 (even samples) / 96 (odd samples); bn1 folded
    into the ReLU eviction's per-partition scale/bias with the channel top-k
    mask (vec in {0,1}) multiplied in. Saliency stays exact f32 because the
    top-k tie margins on these inputs are ~3e-6.
  - channel top-32 mask: exact pairwise greater-counts (tie semantics match
    `sal >= top_k(sal, 32)[-1]` exactly).
  - spatial top-1568 mask: exact 32-step bitwise bisection on the sortable-u32
    transform of the saliency (threshold = bits(kth)-1, mask = u > lo), counts
    aggregated across partitions with a ones-matrix matmul; 2 samples batched;
    the count matmuls own a dedicated PSUM bank so consecutive bisections and
    stage-c work overlap.
  - 3x3 mask dilation: K=9 ones-matmul over 9 shifted copies of the padded
    mask row (built with 3 overlapping-stride DMAs on the gpsimd SWDGE
    queue); applied as min(cnt,1) * r on the DVE.
  - conv2 (3x3) as 9 accumulated K=64 f32r matmuls on a row-padded layout
    (stride 58); two samples share one [128, NP] t12 tile as partition
    halves, 3 tiles rotate so the pipeline runs 3 quads deep.
  - conv3 (1x1) as K=65 f32r matmul: bn3 scale folded into weights, bn3 bias
    applied only at masked pixels via the fused mask row (b3 (x) mask rank-1
    term); identity x re-streamed from DRAM per chunk and added with an f32r
    eye-matmul into the same PSUM; ReLU evicts to bf16 (output tolerance is
    2e-2; bf16 costs ~4e-3) and the host upcasts.

Heavy matmuls run in f32r (1 cycle/row vs 4 for f32 at free-dim >= 448);
producers feeding f32r matmuls are f32r-typed to satisfy the BIR verifier
(DMA moves keep full fp32 bits, compute writers round). Weights arrive
host-pretransposed so all const loads are contiguous. mask_b is ignored:
adding a constant to the saliency cannot change its top-k mask.
"""
import sys

for _p in ("/opt/trn_rl_repo",):
    if _p not in sys.path:
        sys.path.insert(0, _p)

import numpy as np

import concourse.bass as bass
import concourse.tile as tile
from concourse import bacc, mybir

F32 = mybir.dt.float32
F32R = mybir.dt.float32r
U32 = mybir.dt.uint32
OP = mybir.AluOpType
AF = mybir.ActivationFunctionType
AX = mybir.AxisListType
BF16 = mybir.dt.bfloat16

B, CIN, H, W = 64, 256, 56, 56
WIDTH, COUT = 64, 256
N = H * W                      # 3136
K_SP, K_CH = 1568, 32
EPS = 1e-5
NCORES = 8
SPC = B // NCORES              # 8 samples per core

PW = W + 2                     # padded row stride
BASE = 64
NP = BASE + PW * H + BASE      # 3376
CH = 448                       # pixels per chunk (8 rows)
NCH = N // CH                  # 7
RPC = CH // W                  # 8 rows per chunk

UP, UF = 112, 28               # 112*28 == 3136
PAIR = 2


def _padded(t, p0, p1, chunk, off):
    """[p1-p0, 8, 56] view of padded tile t at pixel chunk `chunk` shifted by off."""
    start = BASE + PW * RPC * chunk + off
    return t[p0:p1, start:start + PW * RPC].rearrange("p (h w) -> p h w", h=RPC)[:, :, 0:W]


def _r(ap):
    """Reinterpret an f32 AP as f32r (same bits; tags the value for the PE's
    fast-fp32 mode, 1 cycle/row at free dim >= 256 vs 4 for plain f32).

    f32r is only used where its reduced precision is safe: 0/1-valued count
    matmuls (products exact in any split) and the conv2/conv3/identity data
    path (output tolerance 2e-2). The saliency math (conv1 row 65, fc) stays
    plain f32: top-k tie margins on the fixed inputs are ~3e-6. DMA loads into
    f32r-typed tiles keep full fp32 bits, so f32-bitcast reads of those tiles
    (conv1's rhs) remain exact.
    """
    return ap.bitcast(F32R)


import os
DEBUG = bool(int(os.environ.get("KDEBUG", "0")))
KSKIP = set(os.environ.get("KSKIP", "").split(","))


def _build_nc():
    nc = bacc.Bacc("TRN2", target_bir_lowering=False, debug=False)

    x_d = nc.dram_tensor("x", [SPC, CIN, N], F32, kind="ExternalInput").ap()
    # weights arrive host-pretransposed so every load is contiguous
    c1w_d = nc.dram_tensor("conv1_w", [CIN, WIDTH], F32, kind="ExternalInput").ap()
    bn1 = {k: nc.dram_tensor(f"bn1_{k}", [WIDTH], F32, kind="ExternalInput").ap() for k in "gbmv"}
    c2w_d = nc.dram_tensor("conv2_w", [3, 3, WIDTH, WIDTH], F32, kind="ExternalInput").ap()
    bn2 = {k: nc.dram_tensor(f"bn2_{k}", [WIDTH], F32, kind="ExternalInput").ap() for k in "gbmv"}
    c3w_d = nc.dram_tensor("conv3_w", [WIDTH, COUT], F32, kind="ExternalInput").ap()
    bn3 = {k: nc.dram_tensor(f"bn3_{k}", [COUT], F32, kind="ExternalInput").ap() for k in "gbmv"}
    fcw_d = nc.dram_tensor("fc_w", [CIN, WIDTH], F32, kind="ExternalInput").ap()
    fcb_d = nc.dram_tensor("fc_b", [WIDTH], F32, kind="ExternalInput").ap()
    mw_d = nc.dram_tensor("mask_w", [CIN], F32, kind="ExternalInput").ap()
    nc.dram_tensor("mask_b", [1], F32, kind="ExternalInput")  # unused (constant shift)
    # bf16 output: halves store traffic; quantization error ~0.4% of |y|,
    # far inside the 2e-2 relative gate. Host upcasts to f32.
    y_d = nc.dram_tensor("y", [SPC, COUT, N], BF16, kind="ExternalOutput").ap()

    dbg = {}
    if DEBUG:
        dbg["sal"] = nc.dram_tensor("dbg_sal", [SPC, 64], F32, kind="ExternalOutput").ap()
        dbg["vec"] = nc.dram_tensor("dbg_vec", [SPC, 64], F32, kind="ExternalOutput").ap()
        dbg["sp"] = nc.dram_tensor("dbg_sp", [SPC, N], F32, kind="ExternalOutput").ap()
        dbg["u"] = nc.dram_tensor("dbg_u", [SPC, UP, UF], U32, kind="ExternalOutput").ap()
        dbg["lo"] = nc.dram_tensor("dbg_lo", [SPC, UP], U32, kind="ExternalOutput").ap()
        dbg["mask"] = nc.dram_tensor("dbg_mask", [SPC, N], F32, kind="ExternalOutput").ap()
        dbg["t12"] = nc.dram_tensor("dbg_t12", [SPC, 128, NP], F32, kind="ExternalOutput").ap()
        dbg["rhs65"] = nc.dram_tensor("dbg_rhs65", [SPC, 65, N], F32, kind="ExternalOutput").ap()

    eye128_d = nc.inline_tensor(np.eye(128, dtype=np.float32), "eye128").ap()
    onesum_d = nc.inline_tensor(np.ones((UP, 128), np.float32), "ones_sum").ap()
    ones1x64_d = nc.inline_tensor(np.ones((1, 64), np.float32), "ones1x64").ap()
    ones9_d = nc.inline_tensor(np.ones((9, 64), np.float32), "ones9x64").ap()

    from contextlib import ExitStack
    with tile.TileContext(nc) as tc, ExitStack() as ctx:
        _body(ctx, tc, nc, x_d, y_d, c1w_d, bn1, c2w_d, bn2, c3w_d, bn3,
              fcw_d, fcb_d, mw_d, eye128_d, onesum_d, ones1x64_d, ones9_d, dbg)
    nc.compile()
    return nc


def _body(ctx, tc, nc, x_d, y_d, c1w_d, bn1, c2w_d, bn2, c3w_d, bn3,
          fcw_d, fcb_d, mw_d, eye128_d, onesum_d, ones1x64_d, ones9_d, dbg):
    consts = ctx.enter_context(tc.tile_pool(name="consts", bufs=1))
    xpool = ctx.enter_context(tc.tile_pool(name="xp", bufs=4))
    # conv3 identity chunks re-loaded from DRAM (frees x tiles right after
    # stage_a so the next quad's x loads overlap the bisection)
    xcp = ctx.enter_context(tc.tile_pool(name="xc", bufs=6))
    statics = ctx.enter_context(tc.tile_pool(name="statics", bufs=2))
    rhs65p = ctx.enter_context(tc.tile_pool(name="rhs65", bufs=2))
    rowp = ctx.enter_context(tc.tile_pool(name="rows", bufs=1))
    smallp = ctx.enter_context(tc.tile_pool(name="smalls", bufs=5))
    upool = ctx.enter_context(tc.tile_pool(name="utiles", bufs=4))
    outp = ctx.enter_context(tc.tile_pool(name="outs", bufs=3))
    # PSUM budget is 8 banks (one matmul-output tile each, 2KB zero regions).
    # z1 doubles as the ring for stage-a1's small outputs (tag "z1"), freeing
    # a dedicated bank for the bisection counts so bisect(q) never shares a
    # ring with stage_c(q-1)'s dilation/mask matmuls (that sharing serialized
    # the whole machine around each bisect).
    ps_z1 = ctx.enter_context(tc.tile_pool(name="ps_z1", bufs=2, space="PSUM"))
    ps_z2 = ctx.enter_context(tc.tile_pool(name="ps_z2", bufs=2, space="PSUM"))
    ps_z3 = ctx.enter_context(tc.tile_pool(name="ps_z3", bufs=2, space="PSUM"))
    ps_bis = ctx.enter_context(tc.tile_pool(name="ps_bis", bufs=2, space="PSUM"))
    ps_cnt = ps_z2
    ps_sm = ps_z1

    # first quad's x tiles load before the ~45 const DMAs so they don't
    # queue behind them on the HWDGE generator (ready-heap prefers emission
    # order)
    xearly = {}
    for _s in range(PAIR):
        _ts = []
        for _k in range(2):
            _xt = xpool.tile([128, N], F32, name=f"x{_k}_s{_s}", tag="x")
            nc.sync.dma_start(_xt, x_d[_s, 128 * _k:128 * (_k + 1)])
            _ts.append(_xt)
        xearly[_s] = _ts

    # ---------- constants ----------
    ident = consts.tile([128, 128], F32)
    nc.sync.dma_start(ident, eye128_d)
    identr = consts.tile([128, 128], F32R)
    nc.sync.dma_start(identr, _r(eye128_d))
    onesum = consts.tile([UP, 128], F32)
    nc.sync.dma_start(onesum, onesum_d)
    ones1x64 = consts.tile([1, 64], F32R)
    nc.sync.dma_start(ones1x64, _r(ones1x64_d))
    # lives at partitions 32-40 to match the msh shift rows (matmul requires
    # equal base partitions on both operands)
    ones9t = consts.tile([41, 64], F32R, name="ones9t")
    nc.sync.dma_start(ones9t[32:41], _r(ones9_d))
    ones9 = ones9t[32:41]

    # u32 bit-pattern constant columns (immediates >= 2^31 are unreliable)
    bits = consts.tile([UP, 33], U32)
    for k in range(32):
        nc.vector.memset(bits[:, k:k + 1], 1 << k)
    nc.vector.memset(bits[:, 32:33], 0x80000000)


    # conv1 lhsT: two [128, 65] K-tiles; col 64 = mask_w
    # cols 64 AND 65 both carry mask_w: even samples read their saliency from
    # output partition 64, odd from 65, so both parities share one sprow tile
    # without a write-after-read hazard
    # cols 64 AND 96 both carry mask_w (engine base partitions must be 0 mod
    # 32): even samples read their saliency from output partition 64, odd
    # from 96, so both parities share one sprow tile without a WAR hazard
    w1 = []
    for k in range(2):
        t = consts.tile([128, 97], F32, name=f"w1_{k}")
        nc.vector.memset(t, 0.0)
        nc.sync.dma_start(t[:, 0:64], c1w_d[128 * k:128 * (k + 1), :])
        nc.sync.dma_start(t[:, 64:65], mw_d[128 * k:128 * (k + 1)].unsqueeze(1))
        nc.sync.dma_start(t[:, 96:97], mw_d[128 * k:128 * (k + 1)].unsqueeze(1))
        w1.append(t)

    # fc lhsT: two [128, 64] K-tiles; fc_b as [64,1]
    fcw = []
    for k in range(2):
        t = consts.tile([128, 64], F32, name=f"fcw_{k}")
        nc.sync.dma_start(t, fcw_d[128 * k:128 * (k + 1), :])
        fcw.append(t)
    fcb_col = consts.tile([64, 1], F32)
    nc.sync.dma_start(fcb_col, fcb_d.unsqueeze(1))

    # conv2 taps
    def tap_ap(dy, dx):
        return c2w_d[dy + 1, dx + 1]

    # 9 single K=64 taps (no packed pairs): costs 3 extra f32r matmuls per
    # chunk but kills the per-sample t12 shift DMA and lets two samples share
    # one [128, NP] t12 tile as partition halves. Each tap is stored twice
    # (partitions 0-63 and 64-127) so lhsT base matches either t12 half.
    w2t = []
    for dy in (-1, 0, 1):
        for dx in (-1, 0, 1):
            t = consts.tile([128, 64], F32R, name=f"w2_{dy + 1}{dx + 1}")
            nc.sync.dma_start(t[0:64], _r(tap_ap(dy, dx)))
            nc.sync.dma_start(t[64:128], _r(tap_ap(dy, dx)))
            w2t.append((PW * dy + dx, t))

    eps64 = consts.tile([64, 1], F32)
    nc.vector.memset(eps64, EPS)
    eps2 = consts.tile([2, 1], F32)
    nc.vector.memset(eps2, EPS)

    # bn1 / bn2 scale+bias columns [64,1]
    def bn_prep64(bnd, nm):
        cols = {}
        for k in "gbmv":
            c = smallp.tile([64, 1], F32, name=f"{nm}_{k}", tag=f"{nm}_{k}")
            nc.sync.dma_start(c, bnd[k].unsqueeze(1))
            cols[k] = c
        sd = smallp.tile([64, 1], F32, name=f"{nm}_sd", tag=f"{nm}_sd")
        nc.scalar.activation(sd, cols["v"], AF.Sqrt, bias=eps64, scale=1.0)
        rs = smallp.tile([64, 1], F32, name=f"{nm}_rs", tag=f"{nm}_rs")
        nc.vector.reciprocal(rs, sd)
        s = consts.tile([64, 1], F32, name=f"{nm}_s")
        nc.vector.tensor_mul(s, cols["g"], rs)
        bp = consts.tile([64, 1], F32, name=f"{nm}_bp")
        nc.vector.tensor_mul(bp, cols["m"], s)
        nc.vector.tensor_sub(bp, cols["b"], bp)
        return s, bp

    s1c, b1c = bn_prep64(bn1, "bn1")
    s2c, b2c = bn_prep64(bn2, "bn2")

    # bn3 in [2,128] layout (c = 128*p + f), then conv3 lhsT [65, 256]
    def load_2x128(d, nm):
        t = smallp.tile([2, 128], F32, name=nm, tag=nm)
        nc.sync.dma_start(t, d.rearrange("(p f) -> p f", p=2))
        return t

    g3 = load_2x128(bn3["g"], "g3")
    b3 = load_2x128(bn3["b"], "b3")
    m3 = load_2x128(bn3["m"], "m3")
    v3 = load_2x128(bn3["v"], "v3")
    sd3 = smallp.tile([2, 128], F32, tag="sd3")
    nc.scalar.activation(sd3, v3, AF.Sqrt, bias=eps2, scale=1.0)
    rs3 = smallp.tile([2, 128], F32, tag="rs3")
    nc.vector.reciprocal(rs3, sd3)
    s3 = consts.tile([2, 128], F32)
    nc.vector.tensor_mul(s3, g3, rs3)
    b3p = consts.tile([2, 128], F32)
    nc.vector.tensor_mul(b3p, m3, s3)
    nc.vector.tensor_sub(b3p, b3, b3p)

    w3 = consts.tile([65, 256], F32R)
    nc.sync.dma_start(w3[0:64], _r(c3w_d))
    s3row = consts.tile([1, 256], F32)
    nc.sync.dma_start(s3row, s3)          # [2,128] -> [1,256] partition-major
    nc.sync.dma_start(w3[64:65], _r(b3p))
    s3b = ps_sm.tile([64, 256], F32, tag="z1")
    nc.tensor.matmul(s3b, ones1x64.bitcast(F32), s3row, start=True, stop=True)
    nc.vector.tensor_mul(w3[0:64], w3[0:64].bitcast(F32), s3b)

    # padded statics (pads zeroed once; per-sample writes only touch pixels)
    def zero_f32r(t):
        # memset can't encode f32r; zero the raw bits, then a Copy activation
        # re-types the region as rounded-f32r for the BIR verifier.
        nc.vector.memset(t.bitcast(U32), 0)
        nc.scalar.activation(t, t.bitcast(F32), AF.Copy)

    # 4 logical t12 buffers packed as partition halves of 2 physical tiles
    # (SBUF charges all 128 partitions regardless of tile partition count)
    t12tiles = []
    for i in range(3):
        t = statics.tile([128, NP], F32R, name=f"t12_{i}", tag=f"t12_{i}", bufs=1)
        zero_f32r(t)
        t12tiles.append(t)
    # two mask-row sets so consecutive samples' stage_c can overlap; rows 0-8
    # hold the 9 dilation shifts, row 9 the mask row itself (saves a tile).
    mshs = []
    for i in range(2):
        m = rowp.tile([41, NP], F32R, name=f"msh{i}", tag=f"msh{i}")
        zero_f32r(m)
        mshs.append(m)
    DELTAS = [dy * PW + dx for dy in (-1, 0, 1) for dx in (-1, 0, 1)]

    class S:
        pass

    # ---------------- stage A ----------------
    def load_x(s):
        ts = []
        for k in range(2):
            # plain f32: x feeds the saliency-critical conv1 matmul, and f32r
            # anywhere on this path rounds x (measured 3e-4 saliency error,
            # which flips top-k boundary pixels with ~3e-6 margins).
            xt = xpool.tile([128, N], F32, name=f"x{k}_s{s}", tag="x")
            nc.sync.dma_start(xt, x_d[s, 128 * k:128 * (k + 1)])
            ts.append(xt)
        return ts

    def stage_a(s):
        st = S()
        st.x = xearly.pop(s) if s in xearly else load_x(s)
        if "a1" in KSKIP:
            st.s1v, st.b1v, st.s2v, st.b2v = s1c, b1c, s2c, b2c
        st.sprow = None

        if "a1" not in KSKIP:
            _stage_a1(st, s)
        if "a2" not in KSKIP:
            _stage_a2(st, s)
        if "a3" not in KSKIP:
            _stage_a3(st, s)
        return st

    def _stage_a1(st, s):
        if "a1x" in KSKIP:
            st.s1v, st.b1v, st.s2v, st.b2v = s1c, b1c, s2c, b2c
        # chunked row-sums: 4 short reduces per tile instead of one 3.3us op,
        # so the bisection's latency chain can interleave on the DVE
        pool0 = smallp.tile([128, 1], F32, tag="pool0")
        pool1 = smallp.tile([128, 1], F32, tag="pool1")
        p4a = smallp.tile([128, 4], F32, tag="p4a")
        p4b = smallp.tile([128, 4], F32, tag="p4b")
        for j in range(4):
            nc.vector.reduce_sum(p4a[:, j:j + 1], st.x[0][:, 784 * j:784 * (j + 1)], axis=AX.X)
            nc.vector.reduce_sum(p4b[:, j:j + 1], st.x[1][:, 784 * j:784 * (j + 1)], axis=AX.X)
        nc.vector.reduce_sum(pool0, p4a, axis=AX.X)
        nc.vector.reduce_sum(pool1, p4b, axis=AX.X)
        fcps = ps_sm.tile([64, 1], F32, tag="z1")
        nc.tensor.matmul(fcps, fcw[0], pool0, start=True, stop=False)
        nc.tensor.matmul(fcps, fcw[1], pool1, start=False, stop=True)
        sal = smallp.tile([64, 1], F32, tag="sal")
        nc.scalar.activation(sal, fcps, AF.Sigmoid, bias=fcb_col, scale=1.0 / N)
        if "a1x" in KSKIP:
            return
        salT = ps_sm.tile([1, 64], F32, tag="z1")
        nc.tensor.transpose(salT, sal, ident[0:64, 0:64])
        salrow = smallp.tile([1, 64], F32, tag="salrow")
        nc.scalar.copy(salrow, salT)
        if "a1y" in KSKIP:
            st.s1v, st.b1v, st.s2v, st.b2v = s1c, b1c, s2c, b2c
            return
        aps = ps_sm.tile([64, 64], F32, tag="z1")
        nc.tensor.matmul(aps, ones1x64.bitcast(F32), salrow, start=True, stop=True)
        scr = smallp.tile([64, 64], F32, tag="scr")
        cnt = smallp.tile([64, 1], F32, tag="cnt")
        # in1 must be SBUF: DVE has a single PSUM read port (in0=aps is PSUM)
        nc.vector.scalar_tensor_tensor(scr, aps, sal, sal.broadcast_to([64, 64]),
                                       op0=OP.is_gt, op1=OP.bypass, accum_out=cnt)
        if "a1z" in KSKIP:
            st.s1v, st.b1v, st.s2v, st.b2v = s1c, b1c, s2c, b2c
            return
        vec = smallp.tile([64, 1], F32, tag="vec")
        nc.vector.tensor_scalar(vec, cnt, float(K_CH), None, op0=OP.is_lt)
        if DEBUG:
            nc.sync.dma_start(dbg["sal"][s], sal)
            nc.sync.dma_start(dbg["vec"][s], vec)
        st.s1v = smallp.tile([64, 1], F32, tag="s1v")
        nc.vector.tensor_mul(st.s1v, s1c, vec)
        st.b1v = smallp.tile([64, 1], F32, tag="b1v")
        nc.vector.tensor_mul(st.b1v, b1c, vec)
        st.s2v = smallp.tile([64, 1], F32, tag="s2v")
        nc.vector.tensor_mul(st.s2v, s2c, vec)
        st.b2v = smallp.tile([64, 1], F32, tag="b2v")
        nc.vector.tensor_mul(st.b2v, b2c, vec)

    sprow_sh = rowp.tile([97, N], F32, name="sprow_sh", tag="sprow")

    def _stage_a2(st, s):
        st.t12 = t12tiles[(s // 2) % 3]
        st.tp0 = 64 * (s % 2)          # partition half within the shared tile
        # saliency row stays on partition 64/65 by parity (engine ops cannot
        # cross partitions); both parities share one tile, distinct partitions
        p = 64 + 32 * (s % 2)
        st.sprow_p = p
        for c in range(NCH):
            z1 = ps_z1.tile([97, CH], F32, tag="z1")
            nc.tensor.matmul(z1, w1[0], st.x[0][:, c * CH:(c + 1) * CH],
                             start=True, stop=False)
            nc.tensor.matmul(z1, w1[1], st.x[1][:, c * CH:(c + 1) * CH],
                             start=False, stop=True)
            tv = _padded(st.t12, st.tp0, st.tp0 + 64, c, 0)
            zv = z1[0:64].rearrange("p (h w) -> p h w", h=RPC)
            nc.scalar.activation(tv, zv, AF.Relu, bias=st.b1v, scale=st.s1v)
            nc.scalar.copy(sprow_sh[p:p + 1, c * CH:(c + 1) * CH], z1[p:p + 1])

        st.sprow = sprow_sh

    def _stage_a3(st, s):
        sprow = st.sprow
        # sortable-u32 transform: u = bits ^ (sign ? 0xFFFFFFFF : 0x80000000)
        st.u = upool.tile([UP, UF], U32, name=f"u_s{s}", tag="u")
        nc.gpsimd.dma_start(st.u.bitcast(F32), sprow[st.sprow_p:st.sprow_p + 1])
        if DEBUG:
            nc.sync.dma_start(dbg["sp"][s], sprow[st.sprow_p:st.sprow_p + 1])
        bb = upool.tile([UP, UF], U32, tag="bb")
        nc.vector.tensor_scalar(bb.bitcast(mybir.dt.int32), st.u.bitcast(mybir.dt.int32),
                                31, None, op0=OP.arith_shift_right)
        nc.vector.tensor_tensor(bb, bb, bits[:, 32:33].broadcast_to([UP, UF]),
                                op=OP.bitwise_or)
        nc.vector.tensor_tensor(st.u, st.u, bb, op=OP.bitwise_xor)
        if DEBUG:
            nc.sync.dma_start(dbg["u"][s], st.u)

    # ---------------- bisection (4 samples) ----------------
    def bisect(quad, q):
        lo = upool.tile([UP, PAIR], U32, name=f"lo_q{q}", tag="lo")
        nc.vector.memset(lo, 0)
        mt = upool.tile([UP, PAIR], U32, tag="mt")
        csum = upool.tile([UP, PAIR], F32, tag="csum")
        scr = upool.tile([UP, UF], F32, tag="uscr")
        for bit in range(31, -1, -1):
            nc.vector.tensor_tensor(mt, lo, bits[:, bit:bit + 1].broadcast_to([UP, PAIR]),
                                    op=OP.bitwise_or)
            for i, st in enumerate(quad):
                nc.vector.scalar_tensor_tensor(
                    scr, st.u, 0, mt[:, i:i + 1].broadcast_to([UP, UF]),
                    op0=OP.bypass, op1=OP.is_gt, accum_out=csum[:, i:i + 1])
            cps = ps_bis.tile([128, PAIR], F32, tag="bis")
            nc.tensor.matmul(cps, onesum, csum, start=True, stop=True)
            flag = upool.tile([UP, PAIR], U32, tag="flag")
            nc.vector.tensor_scalar(flag, cps[0:UP], float(K_SP), None, op0=OP.is_ge)
            nc.vector.tensor_scalar(flag, flag, bit, None, op0=OP.logical_shift_left)
            nc.vector.tensor_tensor(lo, lo, flag, op=OP.bitwise_or)
        for i, st in enumerate(quad):
            st.lo, st.lo_i = lo, i
            if DEBUG:
                nc.sync.dma_start(dbg["lo"][q * PAIR + i], lo[:, i:i + 1])

    # ---------------- stage C ----------------
    def stage_c(s, st):
        mtile = upool.tile([UP, UF], F32, tag="mask")
        nc.vector.tensor_tensor(mtile, st.u,
                                st.lo[:, st.lo_i:st.lo_i + 1].broadcast_to([UP, UF]),
                                op=OP.is_gt)
        rhs65 = rhs65p.tile([65, N], F32R, tag="rhs65")
        nc.gpsimd.dma_start(rhs65[64:65], _r(mtile))
        msh = mshs[s % 2]
        mrow = msh[0:1]
        mpad = mrow[:, BASE:BASE + PW * H].rearrange("p (h w) -> p h w", h=H)[:, :, 0:W]
        nc.gpsimd.dma_start(mpad, rhs65[64:65])
        # 9 dilation shifts as 3 DMAs (one per dy) with overlapping dx strides
        L = PW * H
        for i, dy in enumerate((-1, 0, 1)):
            s0 = mrow[:, BASE + dy * PW - 1:BASE + dy * PW - 1 + L]
            src = bass.AP(s0.tensor, s0.offset, [list(s0.ap[0]), [1, 3], [1, L]])
            nc.gpsimd.dma_start(msh[32 + 3 * i:35 + 3 * i, BASE:BASE + L], src)
        if DEBUG:
            nc.sync.dma_start(dbg["mask"][s], rhs65[64:65].bitcast(F32))

        t12 = st.t12
        p0 = st.tp0
        for c in range(NCH):
            cnt9 = ps_cnt.tile([64, CH], F32, tag="z2")
            nc.tensor.matmul(cnt9, ones9,
                             _padded(msh, 32, 41, c, 0), start=True, stop=True)
            tv = _padded(t12, p0, p0 + 64, c, 0)
            cv = cnt9.rearrange("p (h w) -> p h w", h=RPC)
            nc.vector.scalar_tensor_tensor(tv, cv, 1.0,
                                           _padded(t12.bitcast(F32), p0, p0 + 64, c, 0),
                                           op0=OP.min, op1=OP.mult)
        if DEBUG:
            nc.sync.dma_start(dbg["t12"][s], t12.bitcast(F32))

        for c in range(NCH):
            z2 = ps_z2.tile([64, CH], F32, tag="z2")
            for i, (d, wt) in enumerate(w2t):
                nc.tensor.matmul(z2, wt[p0:p0 + 64], _padded(t12, p0, p0 + 64, c, d),
                                 start=(i == 0), stop=(i == 8))
            r2 = outp.tile([64, CH], F32, tag="r2")
            nc.scalar.activation(r2, z2, AF.Relu, bias=st.b2v, scale=st.s2v)
            mbc = ps_cnt.tile([64, CH], F32, tag="z2")
            nc.tensor.matmul(mbc, ones1x64,
                             _padded(mrow, 0, 1, c, 0),
                             start=True, stop=True)
            nc.vector.scalar_tensor_tensor(rhs65[0:64, c * CH:(c + 1) * CH],
                                           mbc, 1.0, r2, op0=OP.bypass, op1=OP.mult)

        if DEBUG:
            nc.sync.dma_start(dbg["rhs65"][s], rhs65.bitcast(F32))
        for c in range(NCH):
            for m in range(2):
                xc = xcp.tile([128, CH], F32R, tag="xc")
                nc.gpsimd.dma_start(xc, _r(x_d[s, 128 * m:128 * (m + 1), c * CH:(c + 1) * CH]))
                z3 = ps_z3.tile([128, CH], F32, tag="z3")
                nc.tensor.matmul(z3, w3[:, 128 * m:128 * (m + 1)],
                                 rhs65[:, c * CH:(c + 1) * CH], start=True, stop=False)
                nc.tensor.matmul(z3, identr, xc, start=False, stop=True)
                ot = outp.tile([128, CH], BF16, tag="ot")
                nc.scalar.activation(ot, z3, AF.Relu)
                nc.sync.dma_start(y_d[s, 128 * m:128 * (m + 1), c * CH:(c + 1) * CH], ot)

    NQ = SPC // PAIR

    def do_bisect(sts, q):
        if "b" not in KSKIP:
            bisect(sts, q)
        else:
            lo = upool.tile([UP, PAIR], U32, tag="lo")
            nc.vector.memset(lo, 0)
            for i, st in enumerate(sts):
                st.lo, st.lo_i = lo, i

    # software-pipelined emission. Ready-instruction priority follows emission
    # order, so each late bisect is emitted AFTER the c-stage meant to fill
    # its serial-chain latency gaps:
    #   a01 b0 | a23 b1 | c0 | a45 | c1 | b2 | a67 | c2 | b3 | c3
    def do_c(q, quads):
        if "c" not in KSKIP:
            for i, st in enumerate(quads[q]):
                stage_c(q * PAIR + i, st)
        del quads[q]

    def do_a(q, quads):
        quads[q] = [stage_a(q * PAIR + i) for i in range(PAIR)]

    quads = {}
    do_a(0, quads)
    do_bisect(quads[0], 0)
    for q in range(NQ):
        if q + 1 < NQ:
            do_a(q + 1, quads)
            do_bisect(quads[q + 1], q + 1)
        do_c(q, quads)


_CACHED = {}
LAST_RESULTS = None


def _get_nc():
    if "nc" not in _CACHED:
        _CACHED["nc"] = _build_nc()
    return _CACHED["nc"]


def kernel(**inputs):
    from concourse.bass_utils import run_bass_kernel_spmd
    nc = _get_nc()
    x = np.ascontiguousarray(np.asarray(inputs["x"], np.float32).reshape(B, CIN, N))
    base = {
        "conv1_w": np.ascontiguousarray(np.asarray(inputs["conv1_w"], np.float32).reshape(WIDTH, CIN).T),
        "conv2_w": np.ascontiguousarray(np.asarray(inputs["conv2_w"], np.float32).transpose(2, 3, 1, 0)),
        "conv3_w": np.ascontiguousarray(np.asarray(inputs["conv3_w"], np.float32).reshape(COUT, WIDTH).T),
        "fc_w": np.ascontiguousarray(np.asarray(inputs["fc_w"], np.float32).T),
        "fc_b": np.ascontiguousarray(np.asarray(inputs["fc_b"], np.float32)),
        "mask_w": np.ascontiguousarray(np.asarray(inputs["mask_w"], np.float32).reshape(CIN)),
        "mask_b": np.ascontiguousarray(np.asarray(inputs["mask_b"], np.float32)),
    }
    for pre in ("bn1", "bn2", "bn3"):
        for k in "gbmv":
            base[f"{pre}_{k}"] = np.ascontiguousarray(np.asarray(inputs[f"{pre}_{k}"], np.float32))
    in_maps = []
    for c in range(NCORES):
        m = dict(base)
        m["x"] = np.ascontiguousarray(x[c * SPC:(c + 1) * SPC])
        in_maps.append(m)
    res = run_bass_kernel_spmd(nc, in_maps, core_ids=list(range(NCORES)))
    global LAST_RESULTS
    LAST_RESULTS = res
    y = np.concatenate([np.asarray(r["y"]).astype(np.float32) for r in res.results], axis=0)
    return y.reshape(B, COUT, H, W)

